# revision 46
# baseline (speedup 1.0000x reference)
"""Grouped-experts SwiGLU kernel for 8 Trainium2 NeuronCores.

Problem: x[E,T,D], w1[E,D,H], w2[E,H,D], w3[E,D,H] with E=8, T=1024,
D=1024, H=2048.  out_e = (silu(x_e @ w1_e) * (x_e @ w3_e)) @ w2_e.

Sharding: expert-parallel, one expert per NeuronCore (E == n_cores == 8).
Each core runs an identical Bass program on its expert's slices; no
collectives are needed and the full output is just the stack of the
per-core outputs.

Internal precision: bf16 operands with fp32 PSUM accumulation
(end-to-end rel err ~4e-3, well inside the 2e-2 gate).  bf16 halves all
HBM traffic and runs matmuls at the full 1 cycle/row PE rate at any
free size.

Host-side marshalling (all inputs, one-time CPU cost): bf16 conversion
plus layouts chosen so every DMA line is contiguous per partition and
x arrives pre-transposed — the TensorEngine does nothing but the
SwiGLU matmuls:
  x     -> [p=128, dd=8, t=1024]       (x[t, dd*128+p])
  w1,w3 -> [p=128, hh=16, dd=8, 128]   (w[dd*128+p, hh*128+j])
  w2    -> [p=128, hh=16, dc=2, 512]   (w2[hh*128+p, dc*512+j])

Per-core schedule:
  0. xT streams in as four token-quarter DMAs on the SP HWDGE ring
     while the first w1/w3 tiles load on the ACT ring; hh=0's stage-A
     units run at 256-token granularity right behind each quarter.
     (On-device transposes were tried both ways and rejected: the xbar
     DMA-transpose serializes the DMA engines on every mode transition,
     and PE-transposes cost 3.4us of PE plus copy/sem pacing stalls.)
  1. Stage A: per H-tile hh, stream w1/w3 slices on the ACT ring,
     accumulate gT/upT = w^T @ x^T over the 8 D-chunks in PSUM, silu on
     ScalarE, multiply on VectorE -> resident bf16 hT (partition = H).
  2. Stage B: w2 prefetches early (delayed via tile_wait_until so it
     does not crowd stage A off the DMA engines); out = h @ w2
     accumulating over the 16 H-chunks; PSUM evacuated via alternating
     Scalar/Vector copies and DMAed out in natural fp32 [T,D] layout.
     The last token tile is split into four 128-wide chains so its
     evacuation pipelines with the final matmuls.

TimelineSim cost-model: 177.0 us makespan, PE busy 167.9 us (pure
matmul; floor 163.6 us at 78.6 TF/s), only a 5.0 us DMA-latency start
gap and a 3.8 us drain tail remain.  HW (axon, For_i slope timing):
rel err 3.6e-3; ~265 us/iteration in quiet windows, tunnel noise can
inflate individual runs.
"""

import sys

if "/opt/trn_rl_repo" not in sys.path:
    sys.path.insert(0, "/opt/trn_rl_repo")

import numpy as np

E, T, D, H = 8, 1024, 1024, 2048
P = 128
NT, ND, NH = T // P, D // P, H // P
TC = 512  # stage-A moving (token) chunk
DC = 512  # stage-B moving (dim) chunk
NTC, NDC = T // TC, D // DC


def build_program(loops: int = 1, reps: int = 1):
    """Build the per-core Bass program.  loops>1 wraps the compute body in
    a hardware For_i loop and reps emits the body that many times inside
    it (for slope timing; consecutive bodies pipeline into each other).
    The result is identical for any (loops, reps)."""
    import concourse.bacc as bacc
    import concourse.mybir as mybir
    from concourse import tile

    f32 = mybir.dt.float32
    bf16 = mybir.dt.bfloat16
    SILU = mybir.ActivationFunctionType.Silu

    nc = bacc.Bacc("TRN2", target_bir_lowering=False, debug=False)
    x_d = nc.declare_dram_parameter("x", [P, ND, T], bf16, isOutput=False)
    w1_d = nc.declare_dram_parameter("w1", [P, NH, ND, P], bf16, isOutput=False)
    w2_d = nc.declare_dram_parameter("w2", [P, NH, NDC, DC], bf16, isOutput=False)
    w3_d = nc.declare_dram_parameter("w3", [P, NH, ND, P], bf16, isOutput=False)
    out_d = nc.declare_dram_parameter("out", [T, D], f32, isOutput=True)

    with tile.TileContext(nc) as tc:
        with (
            tc.tile_pool(name="xT", bufs=1) as xT_pool,
            tc.tile_pool(name="hT", bufs=1) as hT_pool,
            tc.tile_pool(name="wA", bufs=3) as wA_pool,
            tc.tile_pool(name="sg", bufs=3) as sg_pool,
            tc.tile_pool(name="wB", bufs=32) as wB_pool,
            tc.tile_pool(name="ob", bufs=4) as ob_pool,
            tc.tile_pool(name="ps", bufs=8, space="PSUM") as ps_pool,
        ):

            def body():
                # ---- Phase 0: x arrives already transposed ([p, dd, t],
                # host-marshalled like the weights), streamed in four
                # token-quarter DMAs on the SP ring while the first weight
                # tiles load on the ACT ring.  hh=0's stage-A units run at
                # 256-token granularity right behind each quarter; the PE
                # does nothing but SwiGLU matmuls for the whole kernel.
                XQ = T // 4
                xT = xT_pool.tile([P, ND, T], bf16, name="xT", tag="xT")

                def load_wA(hh):
                    w1s = wA_pool.tile([P, ND, P], bf16, name="w1s", tag="w1s")
                    nc.scalar.dma_start(out=w1s[:], in_=w1_d[:, hh])
                    w3s = wA_pool.tile([P, ND, P], bf16, name="w3s", tag="w3s")
                    nc.scalar.dma_start(out=w3s[:], in_=w3_d[:, hh])
                    return w1s, w3s

                for q in range(4):
                    nc.sync.dma_start(
                        out=xT[:, :, q * XQ : (q + 1) * XQ],
                        in_=x_d[:, :, q * XQ : (q + 1) * XQ],
                    )
                wA0 = load_wA(0)

                hT = [
                    hT_pool.tile([P, T], bf16, name=f"hT{hh}", tag=f"hT{hh}")
                    for hh in range(NH)
                ]

                def stageA_unit(hh, w1s, w3s, ts, te):
                    tok = slice(ts, te)
                    g_ps = ps_pool.tile([P, te - ts], f32, name="g_ps", tag="ps")
                    u_ps = ps_pool.tile([P, te - ts], f32, name="u_ps", tag="ps")
                    for dd in range(ND):
                        nc.tensor.matmul(
                            g_ps[:],
                            w1s[:, dd, :],
                            xT[:, dd, tok],
                            start=(dd == 0),
                            stop=(dd == ND - 1),
                        )
                    for dd in range(ND):
                        nc.tensor.matmul(
                            u_ps[:],
                            w3s[:, dd, :],
                            xT[:, dd, tok],
                            start=(dd == 0),
                            stop=(dd == ND - 1),
                        )
                    sg = sg_pool.tile([P, te - ts], f32, name="sg", tag="sg")
                    nc.scalar.activation(sg[:], g_ps[:], SILU)
                    nc.vector.tensor_mul(hT[hh][:, tok], sg[:], u_ps[:])

                # hh=0 runs in 256-token units chasing the x quarters;
                # hh>=1 then runs on full 512-token chunks.
                for q in range(4):
                    stageA_unit(0, *wA0, q * XQ, (q + 1) * XQ)

                # Stage-B weight prefetch, delayed (tile_wait_until) so its
                # 4 MiB doesn't crowd the stage-A weight stream off the DMA
                # engines during the first ~30us; still lands well before
                # stage B consumes it.
                w2s_all = []
                for dc in range(NDC):
                    w2s = []
                    with tc.tile_wait_until(0.045 + 0.030 * dc):
                        for hh in range(NH):
                            w2t = wB_pool.tile(
                                [P, DC], bf16, name="w2t", tag="w2t"
                            )
                            nc.sync.dma_start(out=w2t[:], in_=w2_d[:, hh, dc])
                            w2s.append(w2t)
                    w2s_all.append(w2s)

                # ---- Stage A: hT = silu(w1^T x^T) * (w3^T x^T) ------------
                for hh in range(1, NH):
                    w1s, w3s = load_wA(hh)
                    for c in range(NTC):
                        stageA_unit(hh, w1s, w3s, c * TC, (c + 1) * TC)

                # ---- Stage B: out = h @ w2 --------------------------------
                # The very last token tile is split into two 256-wide
                # accumulation chains so its evacuation pipelines with the
                # final matmuls instead of sitting wholly in the tail.
                for dc in range(NDC):
                    w2s = w2s_all[dc]
                    for t in range(NT):
                        trow = slice(t * P, (t + 1) * P)
                        last = dc == NDC - 1 and t == NT - 1
                        splits = (
                            tuple((i * P, (i + 1) * P) for i in range(DC // P))
                            if last
                            else ((0, DC),)
                        )
                        for si, (cs, ce) in enumerate(splits):
                            o_ps = ps_pool.tile([P, ce - cs], f32, name="o_ps", tag="ps")
                            for hh in range(NH):
                                nc.tensor.matmul(
                                    o_ps[:],
                                    hT[hh][:, trow],
                                    w2s[hh][:, cs:ce],
                                    start=(hh == 0),
                                    stop=(hh == NH - 1),
                                )
                            ob = ob_pool.tile([P, ce - cs], f32, name="ob", tag="ob")
                            if (t + si) % 2 == 0:
                                nc.vector.tensor_copy(ob[:], o_ps[:])
                            else:
                                nc.scalar.copy(ob[:], o_ps[:])
                            eng = nc.sync if (last and si % 2 == 1) else nc.scalar
                            eng.dma_start(
                                out=out_d[trow, dc * DC + cs : dc * DC + ce],
                                in_=ob[:],
                            )

            if loops > 1:
                with tc.For_i(0, loops):
                    for _ in range(reps):
                        body()
            else:
                for _ in range(reps):
                    body()

    nc.compile()
    return nc


_program_cache = {}


def _get_program(loops: int = 1, reps: int = 1):
    if (loops, reps) not in _program_cache:
        _program_cache[(loops, reps)] = build_program(loops, reps)
    return _program_cache[(loops, reps)]


def _to_bf16(a):
    import ml_dtypes

    return np.ascontiguousarray(np.asarray(a, dtype=np.float32)).astype(
        ml_dtypes.bfloat16
    )


def _prep_inputs(x, w1, w2, w3):
    """Convert to bf16 and rearrange into the DMA-friendly layouts the
    program declares (see module docstring)."""
    # x: [E, T, D] -> [E, p, dd, t] with x[e, t, dd*128+p]
    x = np.ascontiguousarray(
        _to_bf16(x).reshape(E, T, ND, P).transpose(0, 3, 2, 1)
    )
    w1 = _to_bf16(w1)
    w2 = _to_bf16(w2)
    w3 = _to_bf16(w3)
    # w1,w3: [E, D, H] -> [E, p, hh, dd, 128] with w[e, dd*128+p, hh*128+j]
    w13 = lambda w: np.ascontiguousarray(
        w.reshape(E, ND, P, NH, P).transpose(0, 2, 3, 1, 4)
    )
    w1r = w13(w1)
    w3r = w13(w3)
    # w2: [E, H, D] -> [E, p, hh, dc, 512] with w2[e, hh*128+p, dc*512+j]
    w2r = np.ascontiguousarray(
        w2.reshape(E, NH, P, NDC, DC).transpose(0, 2, 1, 3, 4)
    )
    return [
        {"x": x[e], "w1": w1r[e], "w2": w2r[e], "w3": w3r[e]} for e in range(E)
    ]


def kernel(x, w1, w2, w3):
    from concourse.bass_utils import run_bass_kernel_spmd

    in_maps = _prep_inputs(x, w1, w2, w3)
    nc = _get_program()
    res = run_bass_kernel_spmd(nc, in_maps, list(range(E)))
    out = np.stack([res.results[e]["out"] for e in range(E)], axis=0)
    return out.astype(np.float32)
